# revision 1
# baseline (speedup 1.0000x reference)
"""GCN (3-layer GraphConv, norm='right') Trainium2 Bass kernel.

Strategy: single NeuronCore, single launch. Per layer:
  gather y[src] rows (256B each) from a DRAM table via dma_gather,
  aggregate per 128-dst-node block with one-hot S-matrix matmuls into PSUM
  (inv_deg folded into S), epilogue applies bias/relu and the next layer's
  projection, writing the next gather table.

Edges are grouped by dst block and split into two streams by src half
(A: src<25088, B: src>=25088) because dma_gather indices are int16.
Per-(block,stream) edge lists are padded to multiples of 128; padding
edges carry slot=999 (matches no dst slot) and inv_deg=0, so they
contribute exactly zero.
"""
import numpy as np

import concourse.bass as bass
import concourse.tile as tile
from concourse import bacc, mybir
from concourse.bass_utils import run_bass_kernel_spmd

N_NODES = 50000
N_EDGES = 800000
IN_FEATS, F, N_CLASSES = 128, 64, 40
NBLK = (N_NODES + 127) // 128          # 391
NROWS = NBLK * 128                     # 50048
HSPLIT = 25088                         # rows [0,HSPLIT) -> stream A
TPC = 16                               # tiles per gather chunk (4096 idxs)
ACT_EVERY = 10 ** 9                    # S-builds stay on VectorE (ACT is slower)

_cache = {}


def _pack_stream(srcv, slotv, invdv, blkv, nblk, base):
    """Pad per-block edge groups to multiples of 128 tiles; return arrays."""
    cnt = np.bincount(blkv, minlength=nblk)
    tiles = (cnt + 127) // 128
    T = int(tiles.sum())
    starts = np.concatenate([[0], np.cumsum(cnt)[:-1]])
    tile_starts = np.concatenate([[0], np.cumsum(tiles)[:-1]])
    idx_pad = np.zeros(T * 128, dtype=np.int16)
    slot_pad = np.full(T * 128, 999.0, dtype=np.float32)
    invd_pad = np.zeros(T * 128, dtype=np.float32)
    if len(srcv):
        rank = np.arange(len(srcv)) - np.repeat(starts, cnt)
        pos = np.repeat(tile_starts * 128, cnt) + rank
        idx_pad[pos] = (srcv - base).astype(np.int16)
        slot_pad[pos] = slotv
        invd_pad[pos] = invdv
    # idx dram layout: index i of the stream at [i%16, i//16], replicated x8
    idx_dram = np.tile(idx_pad.reshape(-1, 16).T, (8, 1)).copy()  # [128, T*8]
    slot_t = slot_pad.reshape(T, 128).T.copy()                    # [128, T]
    invd_t = invd_pad.reshape(T, 128).T.copy()
    return idx_dram, slot_t, invd_t, tiles, tile_starts, T


def _meta3(S):
    """Per-chunk-interleaved [slot | invd] array: [128, 2*T]."""
    T = S[5]
    out = np.empty((128, 2 * max(T, 1)), dtype=np.float32)
    for ch in range((T + TPC - 1) // TPC):
        nt = min(TPC, T - ch * TPC)
        base = 2 * ch * TPC
        out[:, base:base + nt] = S[1][:, ch * TPC:ch * TPC + nt]
        out[:, base + nt:base + 2 * nt] = S[2][:, ch * TPC:ch * TPC + nt]
    return np.ascontiguousarray(out)


def _prep(features, src, dst, W0, b0, W1, b1, W2, b2):
    deg = np.bincount(dst, minlength=N_NODES).astype(np.float32)
    invd = (1.0 / np.maximum(deg, 1.0)).astype(np.float32)

    order = np.argsort(dst, kind="stable")
    dst_s = dst[order].astype(np.int64)
    src_s = src[order].astype(np.int64)
    blk = dst_s // 128
    slot = (dst_s % 128).astype(np.float32)
    invd_e = invd[dst_s]

    am = src_s < HSPLIT
    A = _pack_stream(src_s[am], slot[am], invd_e[am], blk[am], NBLK, 0)
    B = _pack_stream(src_s[~am], slot[~am], invd_e[~am], blk[~am], NBLK, HSPLIT)

    xT = np.zeros((IN_FEATS, NROWS), dtype=np.float32)
    xT[:, :N_NODES] = np.ascontiguousarray(features.T)

    W2p = np.zeros((F, F), dtype=np.float32)
    W2p[:, :N_CLASSES] = W2[:, :N_CLASSES]
    b2p = np.zeros((F, 1), dtype=np.float32)
    b2v = np.asarray(b2).reshape(-1)
    b2p[:min(len(b2v), F), 0] = b2v[:min(len(b2v), F)]

    in_map = {
        "xT": xT,
        "W0": np.ascontiguousarray(W0.astype(np.float32)),
        "W1": np.ascontiguousarray(W1.astype(np.float32)),
        "W2p": W2p,
        "b0": np.asarray(b0, dtype=np.float32).reshape(F, 1),
        "b1": np.asarray(b1, dtype=np.float32).reshape(F, 1),
        "b2p": b2p,
        "iota": np.tile(np.arange(128, dtype=np.float32), (128, 1)),
        "ident": np.eye(128, dtype=np.float32),
        "idxA": A[0], "metaA": _meta3(A),
        "idxB": B[0], "metaB": _meta3(B),
    }
    sched = {"A": (A[3], A[4], A[5]), "B": (B[3], B[4], B[5])}
    return in_map, sched


def _build(sched):
    TA = sched["A"][2]
    TB = sched["B"][2]

    nc = bacc.Bacc("TRN2", num_devices=1, dynamic_dma_scratch_size=65536)
    dt = mybir.dt.float32

    xT_in = nc.dram_tensor("xT", [IN_FEATS, NROWS], dt, kind="ExternalInput")
    W0_in = nc.dram_tensor("W0", [IN_FEATS, F], dt, kind="ExternalInput")
    W1_in = nc.dram_tensor("W1", [F, F], dt, kind="ExternalInput")
    W2_in = nc.dram_tensor("W2p", [F, F], dt, kind="ExternalInput")
    b0_in = nc.dram_tensor("b0", [F, 1], dt, kind="ExternalInput")
    b1_in = nc.dram_tensor("b1", [F, 1], dt, kind="ExternalInput")
    b2_in = nc.dram_tensor("b2p", [F, 1], dt, kind="ExternalInput")
    iota_in = nc.dram_tensor("iota", [128, 128], dt, kind="ExternalInput")
    ident_in = nc.dram_tensor("ident", [128, 128], dt, kind="ExternalInput")
    meta_in = {}
    for s, T in (("A", TA), ("B", TB)):
        meta_in["idx" + s] = nc.dram_tensor("idx" + s, [128, max(T, 1) * 8], mybir.dt.int16, kind="ExternalInput")
        meta_in["meta" + s] = nc.dram_tensor("meta" + s, [128, max(T, 1) * 2], dt, kind="ExternalInput")
    out = nc.dram_tensor("out", [NROWS, F], dt, kind="ExternalOutput")

    with tile.TileContext(nc) as tc:
        with tc.tile_pool(name="const", bufs=1) as cp, \
             tc.tile_pool(name="dram", bufs=1, space="DRAM") as dram, \
             tc.tile_pool(name="msg", bufs=5) as mp, \
             tc.tile_pool(name="midx", bufs=4) as ip, \
             tc.tile_pool(name="marr", bufs=4) as ap_, \
             tc.tile_pool(name="stl", bufs=12) as sp, \
             tc.tile_pool(name="xblk", bufs=4) as xp, \
             tc.tile_pool(name="ep", bufs=4) as epp, \
             tc.tile_pool(name="agg", bufs=3, space="PSUM") as pp, \
             tc.tile_pool(name="eps", bufs=2, space="PSUM") as pp2:

            iota_t = cp.tile([128, 128], dt)
            nc.sync.dma_start(iota_t[:], iota_in[:])
            ident_t = cp.tile([128, 128], dt)
            nc.sync.dma_start(ident_t[:], ident_in[:])
            W0_t = cp.tile([IN_FEATS, F], dt)
            nc.sync.dma_start(W0_t[:], W0_in[:])
            W1_t = cp.tile([F, F], dt)
            nc.sync.dma_start(W1_t[:], W1_in[:])
            W2_t = cp.tile([F, F], dt)
            nc.sync.dma_start(W2_t[:], W2_in[:])
            b0_t = cp.tile([F, 1], dt)
            nc.sync.dma_start(b0_t[:], b0_in[:])
            b1_t = cp.tile([F, 1], dt)
            nc.sync.dma_start(b1_t[:], b1_in[:])
            b2_t = cp.tile([F, 1], dt)
            nc.sync.dma_start(b2_t[:], b2_in[:])

            tbl = []
            for l in range(3):
                tb = dram.tile([NROWS, F], dt, tag=f"t{l}")
                tbl.append(tb)

            # ---- Layer-1 projection: t0 = X @ W0 ----
            for b in range(NBLK):
                xb = xp.tile([IN_FEATS, 128], dt, tag="xb")
                nc.sync.dma_start(xb[:], xT_in[:, b * 128:(b + 1) * 128])
                yp = pp2.tile([128, F], dt, tag="pj")
                nc.tensor.matmul(yp[:], xb[:], W0_t[:], start=True, stop=True)
                ys = epp.tile([128, F], dt, tag="ysb")
                nc.vector.tensor_copy(ys[:], yp[:])
                nc.sync.dma_start(tbl[0][b * 128:(b + 1) * 128, :], ys[:])

            # ---- Layers ----
            tile_ctr = 0
            for l in range(3):
                table = tbl[l]
                views = {"A": table[0:HSPLIT, :], "B": table[HSPLIT:NROWS, :]}
                msgs = {}
                arrs = {}
                for s in ("A", "B"):
                    T = sched[s][2]
                    n_chunks = (T + TPC - 1) // TPC
                    msgs[s] = []
                    arrs[s] = []
                    SC = 4  # chunks per idx/meta load (bigger DMA descriptors)
                    idx_sc = m3_sc = None
                    for ch in range(n_chunks):
                        nt = min(TPC, T - ch * TPC)
                        k = ch % SC
                        if k == 0:
                            ntot = min(SC * TPC, T - ch * TPC)
                            idx_sc = ip.tile([128, ntot * 8], mybir.dt.int16, tag="idx" + s)
                            nc.sync.dma_start(idx_sc[:], meta_in["idx" + s][:, ch * TPC * 8: ch * TPC * 8 + ntot * 8])
                            m3_sc = ap_.tile([128, 2 * ntot], dt, tag="m3" + s)
                            nc.sync.dma_start(m3_sc[:], meta_in["meta" + s][:, 2 * ch * TPC: 2 * ch * TPC + 2 * ntot])
                        idx_t = idx_sc[:, k * TPC * 8: k * TPC * 8 + nt * 8]
                        base = 2 * k * TPC
                        sl = m3_sc[:, base:base + nt]
                        iv = m3_sc[:, base + nt:base + 2 * nt]
                        msg = mp.tile([128, nt, F], dt, tag="msg" + s)
                        nc.gpsimd.dma_gather(
                            msg[:], views[s], idx_t,
                            num_idxs=nt * 128, num_idxs_reg=nt * 128,
                            elem_size=F, single_packet=False)
                        msgs[s].append(msg)
                        arrs[s].append((sl, iv))

                for b in range(NBLK):
                    refs = []
                    for s in ("A", "B"):
                        tiles, tstarts, _T = sched[s]
                        for t in range(int(tstarts[b]), int(tstarts[b] + tiles[b])):
                            refs.append((s, t // TPC, t % TPC))
                    agg = pp.tile([128, F], dt, tag="agg")
                    nt_b = len(refs)
                    for i, (s, ch, col) in enumerate(refs):
                        sl, iv = arrs[s][ch]
                        S = sp.tile([128, 128], dt, tag="S")
                        tile_ctr += 1
                        nc.vector.tensor_scalar(
                            S[:], iota_t[:], sl[:, col:col + 1], iv[:, col:col + 1],
                            mybir.AluOpType.is_equal, mybir.AluOpType.mult)
                        nc.tensor.matmul(agg[:], S[:], msgs[s][ch][:, col, :],
                                         start=(i == 0), stop=(i == nt_b - 1))

                    # epilogue
                    t0 = epp.tile([128, F], dt, tag="t0")
                    if nt_b == 0:
                        nc.vector.memset(t0[:], 0.0)
                    else:
                        nc.vector.tensor_copy(t0[:], agg[:])
                    t0T = pp2.tile([F, 128], dt, tag="t0T")
                    nc.tensor.transpose(t0T[:], t0[:], ident_t[:])
                    rows = slice(b * 128, (b + 1) * 128)
                    if l == 0:
                        hT = epp.tile([F, 128], dt, tag="hT")
                        nc.scalar.activation(hT[:], t0T[:], mybir.ActivationFunctionType.Relu,
                                             bias=b0_t[:, 0:1], scale=1.0)
                        yT = pp2.tile([F, 128], dt, tag="pj")
                        nc.tensor.matmul(yT[:], W1_t[:], hT[:], start=True, stop=True)
                        yTs = epp.tile([F, 128], dt, tag="yTs")
                        nc.vector.tensor_copy(yTs[:], yT[:])
                        yps = pp2.tile([128, F], dt, tag="pj")
                        nc.tensor.transpose(yps[:], yTs[:], ident_t[0:F, 0:F])
                        ysb = epp.tile([128, F], dt, tag="ysb")
                        nc.vector.tensor_copy(ysb[:], yps[:])
                        nc.sync.dma_start(tbl[1][rows, :], ysb[:])
                    elif l == 1:
                        hT = epp.tile([F, 128], dt, tag="hT")
                        nc.scalar.activation(hT[:], t0T[:], mybir.ActivationFunctionType.Relu,
                                             bias=b1_t[:, 0:1], scale=1.0)
                        hps = pp2.tile([128, F], dt, tag="pj")
                        nc.tensor.transpose(hps[:], hT[:], ident_t[0:F, 0:F])
                        hsb = epp.tile([128, F], dt, tag="ysb")
                        nc.vector.tensor_copy(hsb[:], hps[:])
                        nc.sync.dma_start(tbl[2][rows, :], hsb[:])
                    else:
                        # out = aggT.T @ W2p + b2: project the (normalized) agg
                        aT = epp.tile([F, 128], dt, tag="hT")
                        nc.vector.tensor_copy(aT[:], t0T[:])
                        oT = pp2.tile([F, 128], dt, tag="pj")
                        nc.tensor.matmul(oT[:], W2_t[:], aT[:], start=True, stop=True)
                        oTb = epp.tile([F, 128], dt, tag="yTs")
                        nc.scalar.activation(oTb[:], oT[:], mybir.ActivationFunctionType.Identity,
                                             bias=b2_t[:, 0:1], scale=1.0)
                        ops_ = pp2.tile([128, F], dt, tag="pj")
                        nc.tensor.transpose(ops_[:], oTb[:], ident_t[0:F, 0:F])
                        osb = epp.tile([128, F], dt, tag="ysb")
                        nc.vector.tensor_copy(osb[:], ops_[:])
                        nc.sync.dma_start(out[rows, :], osb[:])

    nc.compile()
    return nc


def kernel(features, src, dst, W0, b0, W1, b1, W2, b2):
    features = np.asarray(features, dtype=np.float32)
    src = np.asarray(src).astype(np.int64)
    dst = np.asarray(dst).astype(np.int64)
    in_map, sched = _prep(features, src, dst,
                          np.asarray(W0), np.asarray(b0), np.asarray(W1),
                          np.asarray(b1), np.asarray(W2), np.asarray(b2))
    key = (sched["A"][2], sched["B"][2],
           tuple(sched["A"][0].tolist()), tuple(sched["B"][0].tolist()))
    if _cache.get("key") != key:
        _cache["nc"] = _build(sched)
        _cache["key"] = key
    nc = _cache["nc"]
    res = run_bass_kernel_spmd(nc, [in_map], core_ids=[0])
    full = res.results[0]["out"]
    return np.ascontiguousarray(full[:N_NODES, :N_CLASSES])



# revision 3
# speedup vs baseline: 2.3231x; 2.3231x over previous
"""GCN (3-layer GraphConv, norm='right') Trainium2 Bass kernel — 8-core SPMD.

Sharding: nodes are split into 8 contiguous shards of R=6272 rows (padded to
50176). Core c owns rows [cR,(c+1)R): it holds that shard of the (projected)
feature table and processes every edge whose SRC lies in its shard, so all
gathers are local. Each core accumulates partial aggregates for ALL dst
blocks (one-hot S-matmuls into PSUM, inv_deg folded into S), written
feature-major per block to a partial table [392,64,128]; a ReduceScatter(add)
over the 8 cores then hands core c the complete aggregate for its own 49 dst
blocks (RS chunk boundary == shard boundary). The epilogue applies
bias+ReLU and the next layer's projection, writing the next local table —
no transposes anywhere (aggregation is done transposed: stationary=msg,
moving=S). Layer-3 table is pre-projected by W2 so its aggregation directly
yields logits; b2/8 is added on every core so the RS sum restores b2.

Per-block tile counts are equalized across cores (max over cores) so all 8
cores run the identical program (SPMD); padding edges carry slot=999/invd=0.
"""
import numpy as np

import concourse.bass as bass
import concourse.tile as tile
from concourse import bacc, mybir
from concourse.bass_utils import run_bass_kernel_spmd

N_NODES = 50000
N_EDGES = 800000
IN_FEATS, F, N_CLASSES = 128, 64, 40
NCORES = 8
NBLK = 392                      # dst blocks of 128 rows
NROWS = NBLK * 128              # 50176
BPC = NBLK // NCORES            # 49 blocks per core
R = BPC * 128                   # 6272 rows per core
CH = 48                         # gather chunk size in tiles (128 idxs each)

_cache = {}


def _prep(features, src, dst, W0, b0, W1, b1, W2, b2):
    deg = np.bincount(dst, minlength=N_NODES).astype(np.float32)
    invd = (1.0 / np.maximum(deg, 1.0)).astype(np.float32)

    core = src // R
    blk = dst // 128
    cnt = np.zeros((NCORES, NBLK), dtype=np.int64)
    np.add.at(cnt, (core, blk), 1)
    tiles = np.ceil(cnt.max(axis=0) / 128).astype(np.int64)   # per-block tiles
    tstart = np.concatenate([[0], np.cumsum(tiles)[:-1]])
    T = int(tiles.sum())

    idx_d, slot_d, invd_d = {}, {}, {}
    for c in range(NCORES):
        m = core == c
        s_c = src[m] - c * R
        d_c = dst[m]
        b_c = blk[m]
        order = np.argsort(b_c, kind="stable")
        s_c, d_c, b_c = s_c[order], d_c[order], b_c[order]
        cc = cnt[c]
        starts = np.concatenate([[0], np.cumsum(cc)[:-1]])
        rank = np.arange(len(s_c)) - np.repeat(starts, cc)
        pos = np.repeat(tstart * 128, cc) + rank

        idx_pad = np.zeros(T * 128, dtype=np.int16)
        slot_pad = np.full(T * 128, 999.0, dtype=np.float32)
        invd_pad = np.zeros(T * 128, dtype=np.float32)
        idx_pad[pos] = s_c.astype(np.int16)
        slot_pad[pos] = (d_c % 128).astype(np.float32)
        invd_pad[pos] = invd[d_c]

        idx_d[c] = np.tile(idx_pad.reshape(-1, 16).T, (8, 1)).copy()  # [128,T*8]
        slot_d[c] = np.ascontiguousarray(slot_pad.reshape(T, 128).T)  # [128,T]
        invd_d[c] = np.ascontiguousarray(invd_pad.reshape(T, 128).T)

    W2p = np.zeros((F, F), dtype=np.float16)
    W2p[:, :N_CLASSES] = W2[:, :N_CLASSES].astype(np.float16)
    b2p = np.zeros((F, 1), dtype=np.float32)
    b2p[:min(len(b2), F), 0] = np.asarray(b2).reshape(-1)[:F] / NCORES

    xpad = np.zeros((NROWS, IN_FEATS), dtype=np.float32)
    xpad[:N_NODES] = features
    iota16 = np.tile(np.arange(128, dtype=np.float16), (128, 1))

    in_maps = []
    for c in range(NCORES):
        in_maps.append({
            "xT": np.ascontiguousarray(xpad[c * R:(c + 1) * R].T),
            "idx": idx_d[c], "slot": slot_d[c], "invdm": invd_d[c],
            "W0": np.ascontiguousarray(W0.astype(np.float32)),
            "W1": np.ascontiguousarray(W1.astype(np.float16)),
            "W2p": W2p,
            "b0": np.asarray(b0, dtype=np.float32).reshape(F, 1),
            "b1": np.asarray(b1, dtype=np.float32).reshape(F, 1),
            "b2p": b2p,
            "iota16": iota16,
        })
    return in_maps, (tuple(tiles.tolist()), T)


def _build(tiles, T):
    tstart = np.concatenate([[0], np.cumsum(tiles)[:-1]]).astype(int)
    nch = (T + CH - 1) // CH

    nc = bacc.Bacc("TRN2", num_devices=NCORES, dynamic_dma_scratch_size=65536)
    dt = mybir.dt.float32
    f16 = mybir.dt.float16

    xT_in = nc.dram_tensor("xT", [IN_FEATS, R], dt, kind="ExternalInput")
    idx_in = nc.dram_tensor("idx", [128, T * 8], mybir.dt.int16, kind="ExternalInput")
    slot_in = nc.dram_tensor("slot", [128, T], dt, kind="ExternalInput")
    invd_in = nc.dram_tensor("invdm", [128, T], dt, kind="ExternalInput")
    W0_in = nc.dram_tensor("W0", [IN_FEATS, F], dt, kind="ExternalInput")
    W1_in = nc.dram_tensor("W1", [F, F], f16, kind="ExternalInput")
    W2_in = nc.dram_tensor("W2p", [F, F], f16, kind="ExternalInput")
    b0_in = nc.dram_tensor("b0", [F, 1], dt, kind="ExternalInput")
    b1_in = nc.dram_tensor("b1", [F, 1], dt, kind="ExternalInput")
    b2_in = nc.dram_tensor("b2p", [F, 1], dt, kind="ExternalInput")
    iota_in = nc.dram_tensor("iota16", [128, 128], f16, kind="ExternalInput")
    out = nc.dram_tensor("out", [BPC, N_CLASSES, 128], dt, kind="ExternalOutput")

    with tile.TileContext(nc) as tc:
        with tc.tile_pool(name="const", bufs=1) as cp, \
             tc.tile_pool(name="dram", bufs=1, space="DRAM") as dram, \
             tc.tile_pool(name="msg", bufs=3) as mp, \
             tc.tile_pool(name="msg16", bufs=3) as m16p, \
             tc.tile_pool(name="stl", bufs=8) as sp, \
             tc.tile_pool(name="xblk", bufs=3) as xp, \
             tc.tile_pool(name="ep", bufs=4) as epp, \
             tc.tile_pool(name="agg", bufs=4, space="PSUM") as pp, \
             tc.tile_pool(name="eps", bufs=2, space="PSUM") as pp2:

            W0_t = cp.tile([IN_FEATS, F], dt)
            nc.sync.dma_start(W0_t[:], W0_in[:])
            W1_t = cp.tile([F, F], f16)
            nc.sync.dma_start(W1_t[:], W1_in[:])
            W2_t = cp.tile([F, F], f16)
            nc.sync.dma_start(W2_t[:], W2_in[:])
            b0_t = cp.tile([F, 1], dt)
            nc.sync.dma_start(b0_t[:], b0_in[:])
            b1_t = cp.tile([F, 1], dt)
            nc.sync.dma_start(b1_t[:], b1_in[:])
            b2_t = cp.tile([F, 1], dt)
            nc.sync.dma_start(b2_t[:], b2_in[:])
            iota_t = cp.tile([128, 128], f16)
            nc.sync.dma_start(iota_t[:], iota_in[:])
            idx_t = cp.tile([128, T * 8], mybir.dt.int16)
            nc.sync.dma_start(idx_t[:], idx_in[:])
            slot_t = cp.tile([128, T], dt)
            nc.sync.dma_start(slot_t[:], slot_in[:])
            invd_t = cp.tile([128, T], dt)
            nc.sync.dma_start(invd_t[:], invd_in[:])
            zero16 = cp.tile([F, 128], f16)
            nc.vector.memset(zero16[:], 0.0)

            tbl = [dram.tile([R, F], dt, tag=f"t{l}", name=f"t{l}") for l in range(3)]
            parts = [dram.tile([NBLK, F, 128], f16, tag=f"P{l}", name=f"P{l}") for l in range(2)]
            part3 = dram.tile([NBLK, N_CLASSES, 128], f16, tag="P3")
            rss = [dram.tile([BPC, F, 128], f16, tag=f"rs{l}", name=f"rs{l}") for l in range(2)]
            rs3 = dram.tile([BPC, N_CLASSES, 128], f16, tag="rs3")

            # ---- initial projection: t0 = X_c @ W0 ----
            for k in range(BPC):
                xb = xp.tile([IN_FEATS, 128], dt, tag="xb")
                nc.sync.dma_start(xb[:], xT_in[:, k * 128:(k + 1) * 128])
                pj = pp2.tile([128, F], dt, tag="pj")
                nc.tensor.matmul(pj[:], xb[:], W0_t[:], start=True, stop=True)
                ys = epp.tile([128, F], dt, tag="ys")
                nc.scalar.activation(ys[:], pj[:],
                                     mybir.ActivationFunctionType.Identity)
                nc.sync.dma_start(tbl[0][k * 128:(k + 1) * 128, :], ys[:])

            # ---- layers ----
            for l in range(3):
                table = tbl[l]
                # gather + fp16 conversion, chunked
                msgs = []
                for ch in range(nch):
                    nt = min(CH, T - ch * CH)
                    msg = mp.tile([128, nt, F], dt, tag="msg")
                    nc.gpsimd.dma_gather(
                        msg[:], table[:], idx_t[:, ch * CH * 8: ch * CH * 8 + nt * 8],
                        num_idxs=nt * 128, num_idxs_reg=nt * 128,
                        elem_size=F, single_packet=False)
                    msg16 = m16p.tile([128, nt, F], f16, tag="msg16")
                    nc.scalar.activation(msg16[:], msg[:],
                                         mybir.ActivationFunctionType.Identity)
                    msgs.append(msg16)

                for b in range(NBLK):
                    nt_b = int(tiles[b])
                    if nt_b == 0:
                        if l < 2:
                            nc.sync.dma_start(parts[l][b], zero16[:])
                        else:
                            nc.sync.dma_start(part3[b], zero16[0:N_CLASSES, :])
                        continue
                    aggT = pp.tile([F, 128], dt, tag="aggT")
                    for i in range(nt_b):
                        t = int(tstart[b]) + i
                        S = sp.tile([128, 128], f16, tag="S")
                        nc.vector.tensor_scalar(
                            S[:], iota_t[:], slot_t[:, t:t + 1], invd_t[:, t:t + 1],
                            mybir.AluOpType.is_equal, mybir.AluOpType.mult)
                        nc.tensor.matmul(aggT[:], msgs[t // CH][:, t % CH, :], S[:],
                                         start=(i == 0), stop=(i == nt_b - 1))
                    if l < 2:
                        po = epp.tile([F, 128], f16, tag="po")
                        nc.scalar.activation(po[:], aggT[:],
                                             mybir.ActivationFunctionType.Identity)
                        nc.sync.dma_start(parts[l][b], po[:])
                    else:
                        po = epp.tile([N_CLASSES, 128], f16, tag="po3")
                        nc.scalar.activation(po[:], aggT[0:N_CLASSES, :],
                                             mybir.ActivationFunctionType.Identity,
                                             bias=b2_t[0:N_CLASSES, 0:1])
                        nc.sync.dma_start(part3[b], po[:])

                # ReduceScatter: core c receives its 49 blocks, summed
                if l < 2:
                    nc.gpsimd.collective_compute(
                        "ReduceScatter", mybir.AluOpType.add,
                        replica_groups=[list(range(NCORES))],
                        ins=[parts[l][:]], outs=[rss[l][:]])
                    # epilogue: h = relu(agg + b); next table = h @ W_next
                    bias = b0_t if l == 0 else b1_t
                    Wn = W1_t if l == 0 else W2_t
                    for k in range(BPC):
                        a16 = epp.tile([F, 128], f16, tag="a16")
                        nc.sync.dma_start(a16[:], rss[l][k])
                        hT = epp.tile([F, 128], f16, tag="hT")
                        nc.scalar.activation(hT[:], a16[:],
                                             mybir.ActivationFunctionType.Relu,
                                             bias=bias[:, 0:1])
                        y = pp2.tile([128, F], dt, tag="pj")
                        nc.tensor.matmul(y[:], hT[:], Wn[:], start=True, stop=True)
                        ys = epp.tile([128, F], dt, tag="ys")
                        nc.scalar.activation(ys[:], y[:],
                                             mybir.ActivationFunctionType.Identity)
                        nc.sync.dma_start(tbl[l + 1][k * 128:(k + 1) * 128, :], ys[:])
                else:
                    nc.gpsimd.collective_compute(
                        "ReduceScatter", mybir.AluOpType.add,
                        replica_groups=[list(range(NCORES))],
                        ins=[part3[:]], outs=[rs3[:]])
                    for k in range(BPC):
                        o16 = epp.tile([N_CLASSES, 128], f16, tag="o16")
                        nc.sync.dma_start(o16[:], rs3[k])
                        of = epp.tile([N_CLASSES, 128], dt, tag="of")
                        nc.scalar.activation(of[:], o16[:],
                                             mybir.ActivationFunctionType.Identity)
                        nc.sync.dma_start(out[k], of[:])

    nc.compile()
    return nc


def kernel(features, src, dst, W0, b0, W1, b1, W2, b2):
    features = np.asarray(features, dtype=np.float32)
    src = np.asarray(src).astype(np.int64)
    dst = np.asarray(dst).astype(np.int64)
    in_maps, key = _prep(features, src, dst,
                         np.asarray(W0), np.asarray(b0), np.asarray(W1),
                         np.asarray(b1), np.asarray(W2), np.asarray(b2))
    if _cache.get("key") != key:
        _cache["nc"] = _build(np.asarray(key[0]), key[1])
        _cache["key"] = key
    nc = _cache["nc"]
    res = run_bass_kernel_spmd(nc, in_maps, core_ids=list(range(NCORES)))
    shards = []
    for c in range(NCORES):
        o = res.results[c]["out"]                      # [BPC, 40, 128]
        shards.append(o.transpose(0, 2, 1).reshape(R, N_CLASSES))
    full = np.concatenate(shards, axis=0)
    return np.ascontiguousarray(full[:N_NODES])


# revision 7
# speedup vs baseline: 4.7825x; 2.0586x over previous
"""GCN (3-layer GraphConv, norm='right') Trainium2 Bass kernel — 8-core SPMD.

Sharding: nodes are split into 8 contiguous shards of R=6272 rows (padded to
50176 = 392 blocks of 128). Core c owns rows [cR,(c+1)R): it holds that shard
of the (projected) feature table and processes every edge whose SRC lies in
its shard, so all gathers are local. Each core accumulates partial aggregates
for ALL dst blocks (one-hot S-matmuls into PSUM, inv_deg folded into S, done
transposed: stationary=msg, moving=S, so partials are feature-major and no
PE transposes are ever needed); a ReduceScatter(add) over the 8 cores then
hands core c the complete aggregate for its own 49 dst blocks. The epilogue
applies bias+ReLU and the next layer's projection, writing the next local
table. Layer-3's table is pre-projected by W2 so its aggregation directly
yields logits; b2/8 is added on every core so the RS sum restores b2.

Blocks are processed j-major (round j handles blocks 49c+j for all c) and the
partial table is split [8,24,...] / [8,25,...] so the first ReduceScatter and
its epilogue overlap the second half of aggregation. All small per-block DMAs
are batched 8 blocks at a time (HWDGE fixed cost is ~625ns per descriptor
set); PSUM banks hold 4 aggregation targets each so one ACT copy moves 4
blocks. Per-block tile counts are equalized across cores (max over cores) so
all 8 cores run the identical program (SPMD); padding edges carry slot=999 /
invd=0 and contribute exactly zero.
"""
import numpy as np

import concourse.bass as bass
import concourse.tile as tile
from concourse import bacc, mybir
from concourse.bass_utils import run_bass_kernel_spmd

N_NODES = 50000
N_EDGES = 800000
IN_FEATS, F, N_CLASSES = 128, 64, 40
NCORES = 8
NBLK = 392                      # dst blocks of 128 rows
NROWS = NBLK * 128              # 50176
BPC = NBLK // NCORES            # 49 blocks per core
R = BPC * 128                   # 6272 rows per core
CH = 48                         # gather chunk size in tiles (128 idxs each)
JA = 24                         # rounds in phase A (phase B: BPC-JA = 25)

_cache = {}


def _prep(features, src, dst, W0, b0, W1, b1, W2, b2):
    deg = np.bincount(dst, minlength=N_NODES).astype(np.float32)
    invd = (1.0 / np.maximum(deg, 1.0)).astype(np.float32)

    core = src // R
    blk = dst // 128
    cnt = np.zeros((NCORES, NBLK), dtype=np.int64)
    np.add.at(cnt, (core, blk), 1)
    tiles = np.maximum(np.ceil(cnt.max(axis=0) / 128).astype(np.int64), 1)
    # j-major processing order: round j handles blocks 49c+j for c=0..7
    order = np.array([BPC * c + j for j in range(BPC) for c in range(NCORES)])
    tstart = np.zeros(NBLK, dtype=np.int64)
    tstart[order] = np.concatenate([[0], np.cumsum(tiles[order])[:-1]])
    T = int(tiles.sum())

    idx_d, slot_d, invd_d = {}, {}, {}
    for c in range(NCORES):
        m = core == c
        s_c = src[m] - c * R
        d_c = dst[m]
        b_c = blk[m]
        o = np.argsort(b_c, kind="stable")
        s_c, d_c, b_c = s_c[o], d_c[o], b_c[o]
        cc = cnt[c]
        starts = np.concatenate([[0], np.cumsum(cc)[:-1]])
        rank = np.arange(len(s_c)) - np.repeat(starts, cc)
        # position: tstart of the edge's block (in tile units) * 128 + rank
        pos = tstart[b_c] * 128 + rank

        idx_pad = np.zeros(T * 128, dtype=np.int16)
        slot_pad = np.full(T * 128, 999.0, dtype=np.float32)
        invd_pad = np.zeros(T * 128, dtype=np.float32)
        idx_pad[pos] = s_c.astype(np.int16)
        slot_pad[pos] = (d_c % 128).astype(np.float32)
        invd_pad[pos] = invd[d_c]

        idx_d[c] = np.tile(idx_pad.reshape(-1, 16).T, (8, 1)).copy()  # [128,T*8]
        slot_d[c] = np.ascontiguousarray(slot_pad.reshape(T, 128).T)  # [128,T]
        invd_d[c] = np.ascontiguousarray(invd_pad.reshape(T, 128).T)

    W2p = np.zeros((F, F), dtype=np.float16)
    W2p[:, :N_CLASSES] = W2[:, :N_CLASSES].astype(np.float16)
    b2p = np.zeros((F, 1), dtype=np.float32)
    b2p[:min(len(b2), F), 0] = np.asarray(b2).reshape(-1)[:F] / NCORES

    xpad = np.zeros((NROWS, IN_FEATS), dtype=np.float32)
    xpad[:N_NODES] = features
    iota16 = np.tile(np.arange(128, dtype=np.float16), (128, 1))

    in_maps = []
    for c in range(NCORES):
        in_maps.append({
            "xT": np.ascontiguousarray(xpad[c * R:(c + 1) * R].T),
            "idx": idx_d[c], "slot": slot_d[c], "invdm": invd_d[c],
            "W0": np.ascontiguousarray(W0.astype(np.float32)),
            "W1": np.ascontiguousarray(W1.astype(np.float16)),
            "W2p": W2p,
            "b0": np.asarray(b0, dtype=np.float32).reshape(F, 1),
            "b1": np.asarray(b1, dtype=np.float32).reshape(F, 1),
            "b2p": b2p,
            "iota16": iota16,
        })
    return in_maps, (tuple(tiles.tolist()), T)


def _build(tiles, T):
    nch = (T + CH - 1) // CH

    nc = bacc.Bacc("TRN2", num_devices=NCORES, dynamic_dma_scratch_size=65536)
    dt = mybir.dt.float32
    f16 = mybir.dt.float16

    xT_in = nc.dram_tensor("xT", [IN_FEATS, R], dt, kind="ExternalInput")
    idx_in = nc.dram_tensor("idx", [128, T * 8], mybir.dt.int16, kind="ExternalInput")
    slot_in = nc.dram_tensor("slot", [128, T], dt, kind="ExternalInput")
    invd_in = nc.dram_tensor("invdm", [128, T], dt, kind="ExternalInput")
    W0_in = nc.dram_tensor("W0", [IN_FEATS, F], dt, kind="ExternalInput")
    W1_in = nc.dram_tensor("W1", [F, F], f16, kind="ExternalInput")
    W2_in = nc.dram_tensor("W2p", [F, F], f16, kind="ExternalInput")
    b0_in = nc.dram_tensor("b0", [F, 1], dt, kind="ExternalInput")
    b1_in = nc.dram_tensor("b1", [F, 1], dt, kind="ExternalInput")
    b2_in = nc.dram_tensor("b2p", [F, 1], dt, kind="ExternalInput")
    iota_in = nc.dram_tensor("iota16", [128, 128], f16, kind="ExternalInput")
    out = nc.dram_tensor("out", [BPC, N_CLASSES, 128], dt, kind="ExternalOutput")

    JB = BPC - JA

    with tile.TileContext(nc) as tc:
        with tc.tile_pool(name="const", bufs=1) as cp, \
             tc.tile_pool(name="dram", bufs=1, space="DRAM") as dram, \
             tc.tile_pool(name="msg", bufs=3) as mp, \
             tc.tile_pool(name="msg16", bufs=3) as m16p, \
             tc.tile_pool(name="stl", bufs=10) as sp, \
             tc.tile_pool(name="xbp", bufs=2) as xbp, \
             tc.tile_pool(name="po", bufs=3) as pop, \
             tc.tile_pool(name="ep", bufs=2) as epp, \
             tc.tile_pool(name="agg", bufs=4, space="PSUM") as pp, \
             tc.tile_pool(name="eps", bufs=2, space="PSUM") as pp2:

            W0_t = cp.tile([IN_FEATS, F], dt)
            nc.sync.dma_start(W0_t[:], W0_in[:])
            W1_t = cp.tile([F, F], f16)
            nc.sync.dma_start(W1_t[:], W1_in[:])
            W2_t = cp.tile([F, F], f16)
            nc.sync.dma_start(W2_t[:], W2_in[:])
            b0_t = cp.tile([F, 1], dt)
            nc.sync.dma_start(b0_t[:], b0_in[:])
            b1_t = cp.tile([F, 1], dt)
            nc.sync.dma_start(b1_t[:], b1_in[:])
            b2_t = cp.tile([F, 1], dt)
            nc.sync.dma_start(b2_t[:], b2_in[:])
            iota_t = cp.tile([128, 128], f16)
            nc.sync.dma_start(iota_t[:], iota_in[:])
            idx_t = cp.tile([128, T * 8], mybir.dt.int16)
            nc.sync.dma_start(idx_t[:], idx_in[:])
            slot_t = cp.tile([128, T], dt)
            nc.sync.dma_start(slot_t[:], slot_in[:])
            invd_t = cp.tile([128, T], dt)
            nc.sync.dma_start(invd_t[:], invd_in[:])

            tbl = [dram.tile([R, F], dt, tag=f"t{l}", name=f"t{l}") for l in range(3)]
            pA = [dram.tile([NCORES, JA, F, 128], f16, tag=f"pA{l}", name=f"pA{l}")
                  for l in range(2)]
            pB = [dram.tile([NCORES, JB, F, 128], f16, tag=f"pB{l}", name=f"pB{l}")
                  for l in range(2)]
            p3A = dram.tile([NCORES, JA, N_CLASSES, 128], f16, tag="p3A")
            p3B = dram.tile([NCORES, JB, N_CLASSES, 128], f16, tag="p3B")
            rsA = [dram.tile([JA, F, 128], f16, tag=f"rsA{l}", name=f"rsA{l}")
                   for l in range(2)]
            rsB = [dram.tile([JB, F, 128], f16, tag=f"rsB{l}", name=f"rsB{l}")
                   for l in range(2)]
            rs3A = dram.tile([JA, N_CLASSES, 128], f16, tag="rs3A")
            rs3B = dram.tile([JB, N_CLASSES, 128], f16, tag="rs3B")

            # ---- initial projection: t0 = X_c @ W0, batched 8 blocks ----
            for k0 in range(0, BPC, 8):
                g = min(8, BPC - k0)
                xb = xbp.tile([IN_FEATS, 8, 128], dt, tag="xb")
                nc.sync.dma_start(xb[:, 0:g, :].rearrange("f g s -> f (g s)"),
                                  xT_in[:, k0 * 128:(k0 + g) * 128])
                pj = pp2.tile([128, 8, F], dt, tag="pj8")
                for i in range(g):
                    nc.tensor.matmul(pj[:, i, :], xb[:, i, :], W0_t[:],
                                     start=True, stop=True)
                ys = epp.tile([128, 8, F], dt, tag="ys8")
                nc.scalar.activation(ys[:, 0:g, :], pj[:, 0:g, :],
                                     mybir.ActivationFunctionType.Identity)
                nc.sync.dma_start(
                    tbl[0][k0 * 128:(k0 + g) * 128, :].rearrange(
                        "(g p) f -> p g f", g=g), ys[:, 0:g, :])

            # ---- layers ----
            def epilogue(l, jlo, jhi, rs):
                """Post-RS work for rounds [jlo,jhi): bias+relu+project or
                final output conversion."""
                if l < 2:
                    bias = b0_t if l == 0 else b1_t
                    Wn = W1_t if l == 0 else W2_t
                    for k0 in range(jlo, jhi, 8):
                        g = min(8, jhi - k0)
                        a16 = epp.tile([F, 8, 128], f16, tag="a16")
                        nc.sync.dma_start(
                            a16[:, 0:g, :],
                            rs[k0 - jlo:k0 - jlo + g].rearrange("g f s -> f g s"))
                        hT = epp.tile([F, 8, 128], f16, tag="hT")
                        nc.scalar.activation(hT[:, 0:g, :], a16[:, 0:g, :],
                                             mybir.ActivationFunctionType.Relu,
                                             bias=bias[:, 0:1])
                        y8 = pp2.tile([128, 8, F], dt, tag="pj8")
                        for i in range(g):
                            nc.tensor.matmul(y8[:, i, :], hT[:, i, :], Wn[:],
                                             start=True, stop=True)
                        ys = epp.tile([128, 8, F], dt, tag="ys8")
                        nc.scalar.activation(ys[:, 0:g, :], y8[:, 0:g, :],
                                             mybir.ActivationFunctionType.Identity)
                        nc.sync.dma_start(
                            tbl[l + 1][k0 * 128:(k0 + g) * 128, :].rearrange(
                                "(g p) f -> p g f", g=g), ys[:, 0:g, :])
                else:
                    for k0 in range(jlo, jhi, 8):
                        g = min(8, jhi - k0)
                        o16 = epp.tile([N_CLASSES, 8, 128], f16, tag="o16")
                        nc.sync.dma_start(
                            o16[:, 0:g, :],
                            rs[k0 - jlo:k0 - jlo + g].rearrange("g f s -> f g s"))
                        of = epp.tile([N_CLASSES, 8, 128], dt, tag="of")
                        nc.scalar.activation(of[:, 0:g, :], o16[:, 0:g, :],
                                             mybir.ActivationFunctionType.Identity)
                        nc.sync.dma_start(
                            out[k0:k0 + g].rearrange("g f s -> f g s"),
                            of[:, 0:g, :])

            for l in range(3):
                table = tbl[l]
                msgs = []          # per chunk: msg16 tile
                tile_ptr = 0

                def emit_chunk():
                    ch = len(msgs)
                    nt = min(CH, T - ch * CH)
                    msg = mp.tile([128, CH, F], dt, tag="msg")
                    nc.gpsimd.dma_gather(
                        msg[:, 0:nt, :], table[:],
                        idx_t[:, ch * CH * 8: ch * CH * 8 + nt * 8],
                        num_idxs=nt * 128, num_idxs_reg=nt * 128,
                        elem_size=F, single_packet=False)
                    msg16 = m16p.tile([128, CH, F], f16, tag="msg16")
                    nc.scalar.activation(msg16[:, 0:nt, :], msg[:, 0:nt, :],
                                         mybir.ActivationFunctionType.Identity)
                    msgs.append(msg16)

                pend = []          # deferred epilogue phases
                for j in range(BPC):
                    if j == JA:
                        if l < 2:
                            nc.gpsimd.collective_compute(
                                "ReduceScatter", mybir.AluOpType.add,
                                replica_groups=[list(range(NCORES))],
                                ins=[pA[l][:]], outs=[rsA[l][:]])
                            pend.append((0, JA, rsA[l]))
                        else:
                            nc.gpsimd.collective_compute(
                                "ReduceScatter", mybir.AluOpType.add,
                                replica_groups=[list(range(NCORES))],
                                ins=[p3A[:]], outs=[rs3A[:]])
                            pend.append((0, JA, rs3A))
                    if j == JA + 11 and pend:
                        epilogue(l, *pend.pop())
                    ph_f = F if l < 2 else N_CLASSES
                    po = pop.tile([ph_f, NCORES, 128], f16, tag="po")
                    for half in range(2):
                        agg = pp.tile([F, 4, 128], dt, tag="agg")
                        for ci in range(4):
                            c = half * 4 + ci
                            nt_b = int(tiles[BPC * c + j])
                            for i in range(nt_b):
                                t = tile_ptr
                                tile_ptr += 1
                                if t >= len(msgs) * CH:
                                    emit_chunk()
                                S = sp.tile([128, 128], f16, tag="S")
                                nc.vector.tensor_scalar(
                                    S[:], iota_t[:], slot_t[:, t:t + 1],
                                    invd_t[:, t:t + 1],
                                    mybir.AluOpType.is_equal, mybir.AluOpType.mult)
                                nc.tensor.matmul(
                                    agg[:, ci, :], msgs[t // CH][:, t % CH, :], S[:],
                                    start=(i == 0), stop=(i == nt_b - 1))
                        if l < 2:
                            nc.scalar.activation(
                                po[:, half * 4:half * 4 + 4, :], agg[:],
                                mybir.ActivationFunctionType.Identity)
                        else:
                            nc.scalar.activation(
                                po[:, half * 4:half * 4 + 4, :],
                                agg[0:N_CLASSES, :, :],
                                mybir.ActivationFunctionType.Identity,
                                bias=b2_t[0:N_CLASSES, 0:1])
                    if l < 2:
                        dst = pA[l][:, j] if j < JA else pB[l][:, j - JA]
                    else:
                        dst = p3A[:, j] if j < JA else p3B[:, j - JA]
                    nc.sync.dma_start(dst.rearrange("g f s -> f g s"), po[:])

                if l < 2:
                    nc.gpsimd.collective_compute(
                        "ReduceScatter", mybir.AluOpType.add,
                        replica_groups=[list(range(NCORES))],
                        ins=[pB[l][:]], outs=[rsB[l][:]])
                    while pend:
                        epilogue(l, *pend.pop())
                    epilogue(l, JA, BPC, rsB[l])
                else:
                    nc.gpsimd.collective_compute(
                        "ReduceScatter", mybir.AluOpType.add,
                        replica_groups=[list(range(NCORES))],
                        ins=[p3B[:]], outs=[rs3B[:]])
                    while pend:
                        epilogue(l, *pend.pop())
                    epilogue(l, JA, BPC, rs3B)

    nc.compile()
    return nc


def kernel(features, src, dst, W0, b0, W1, b1, W2, b2):
    features = np.asarray(features, dtype=np.float32)
    src = np.asarray(src).astype(np.int64)
    dst = np.asarray(dst).astype(np.int64)
    in_maps, key = _prep(features, src, dst,
                         np.asarray(W0), np.asarray(b0), np.asarray(W1),
                         np.asarray(b1), np.asarray(W2), np.asarray(b2))
    if _cache.get("key") != key:
        _cache["nc"] = _build(np.asarray(key[0]), key[1])
        _cache["key"] = key
    nc = _cache["nc"]
    res = run_bass_kernel_spmd(nc, in_maps, core_ids=list(range(NCORES)))
    shards = []
    for c in range(NCORES):
        o = res.results[c]["out"]                      # [BPC, 40, 128]
        shards.append(o.transpose(0, 2, 1).reshape(R, N_CLASSES))
    full = np.concatenate(shards, axis=0)
    return np.ascontiguousarray(full[:N_NODES])


# revision 8
# speedup vs baseline: 5.6933x; 1.1904x over previous
"""GCN (3-layer GraphConv, norm='right') Trainium2 Bass kernel — 8-core SPMD.

Sharding: nodes are split into 8 contiguous shards of R=6272 rows (padded to
50176 = 392 blocks of 128). Core c owns rows [cR,(c+1)R): it holds that shard
of the (projected) feature table and processes every edge whose SRC lies in
its shard, so all gathers are local. Each core accumulates partial aggregates
for ALL dst blocks (one-hot S-matmuls into PSUM, inv_deg folded into S, done
transposed: stationary=msg, moving=S, so partials are feature-major and no
PE transposes are ever needed); ReduceScatter(add) over the 8 cores then
hands core c the complete aggregate for its own 49 dst blocks. The epilogue
applies bias+ReLU and the next layer's projection, writing the next local
table. Layer-3's table is pre-projected by W2 so its aggregation directly
yields logits; b2/8 is added on every core so the RS sum restores b2.

Edge layout (straddle): blocks are processed j-major (round j = blocks 49c+j
for all c) and each block's edge region is sized max_c cnt[c,b] — regions are
packed contiguously WITHOUT per-block 128-alignment, so a 128-edge gather
tile may straddle adjacent block regions; each (tile, block-overlap) gets its
own one-hot S matmul (pair). This keeps gather padding at ~9% instead of
~50%. Gathered rows cost one DMA descriptor each (~1.42ns effective), making
rows-gathered the dominant term; everything else (S-builds on DVE, fp16
matmuls on PE, copies on ACT, partial writes) overlaps under it.

The per-layer partial table is split into three j-ranges so the first two
ReduceScatters (and their epilogues) overlap the remaining aggregation; only
the last small RS sits on the layer tail. All small DMAs are batched 8
blocks at a time (HWDGE fixed cost ~625ns each); PSUM holds 8 aggregation
targets per 2-bank tile so one ACT copy moves a whole round. Region sizes
are equalized across cores (max over cores) so all 8 cores run the identical
program (SPMD); padding edges carry slot=999 and contribute exactly zero.
"""
import numpy as np

import concourse.bass as bass
import concourse.tile as tile
from concourse import bacc, mybir
from concourse.bass_utils import run_bass_kernel_spmd

N_NODES = 50000
N_EDGES = 800000
IN_FEATS, F, N_CLASSES = 128, 64, 40
NCORES = 8
NBLK = 392                      # dst blocks of 128 rows
NROWS = NBLK * 128              # 50176
BPC = NBLK // NCORES            # 49 blocks per core
R = BPC * 128                   # 6272 rows per core
CH = 48                         # gather chunk size in tiles (128 idxs each)
JS = (0, 24, 40, BPC)           # ReduceScatter split points (rounds)

_cache = {}


def _prep(features, src, dst, W0, b0, W1, b1, W2, b2):
    deg = np.bincount(dst, minlength=N_NODES).astype(np.float32)
    invd = (1.0 / np.maximum(deg, 1.0)).astype(np.float32)

    core = src // R
    blk = dst // 128
    cnt = np.zeros((NCORES, NBLK), dtype=np.int64)
    np.add.at(cnt, (core, blk), 1)
    nb = np.maximum(cnt.max(axis=0), 1)              # region size per block

    # processing order: j-major
    order = np.array([BPC * c + j for j in range(BPC) for c in range(NCORES)])
    estart = np.zeros(NBLK, dtype=np.int64)
    estart[order] = np.concatenate([[0], np.cumsum(nb[order])[:-1]])
    E_tot = int(nb.sum())
    T = (E_tot + 127) // 128

    t_lo = estart // 128
    t_hi = (estart + nb - 1) // 128
    npairs = (t_hi - t_lo + 1).astype(np.int64)
    pair_base = np.zeros(NBLK, dtype=np.int64)
    pair_base[order] = np.concatenate([[0], np.cumsum(npairs[order])[:-1]])
    NP = int(npairs.sum())
    pair_tile = np.zeros(NP, dtype=np.int64)
    for b in range(NBLK):
        pair_tile[pair_base[b]:pair_base[b] + npairs[b]] = np.arange(
            t_lo[b], t_hi[b] + 1)

    idx_d, slotp_d, invd_d = {}, {}, {}
    for c in range(NCORES):
        m = core == c
        s_c = src[m] - c * R
        d_c = dst[m]
        b_c = blk[m]
        o = np.argsort(b_c, kind="stable")
        s_c, d_c, b_c = s_c[o], d_c[o], b_c[o]
        cc = cnt[c]
        starts = np.concatenate([[0], np.cumsum(cc)[:-1]])
        rank = np.arange(len(s_c)) - np.repeat(starts, cc)
        pos = estart[b_c] + rank
        tile_e = pos // 128
        lane = pos % 128

        idx_pad = np.zeros(T * 128, dtype=np.int16)
        invd_pad = np.zeros(T * 128, dtype=np.float32)
        idx_pad[pos] = s_c.astype(np.int16)
        invd_pad[pos] = invd[d_c]
        slotp = np.full((128, NP), 999.0, dtype=np.float32)
        slotp[lane, pair_base[b_c] + tile_e - t_lo[b_c]] = (
            d_c % 128).astype(np.float32)

        idx_d[c] = np.tile(idx_pad.reshape(-1, 16).T, (8, 1)).copy()  # [128,T*8]
        invd_d[c] = np.ascontiguousarray(invd_pad.reshape(T, 128).T)  # [128,T]
        slotp_d[c] = slotp

    W2p = np.zeros((F, F), dtype=np.float16)
    W2p[:, :N_CLASSES] = W2[:, :N_CLASSES].astype(np.float16)
    b2p = np.zeros((F, 1), dtype=np.float32)
    b2p[:min(len(b2), F), 0] = np.asarray(b2).reshape(-1)[:F] / NCORES

    xpad = np.zeros((NROWS, IN_FEATS), dtype=np.float32)
    xpad[:N_NODES] = features
    iota16 = np.tile(np.arange(128, dtype=np.float16), (128, 1))

    in_maps = []
    for c in range(NCORES):
        in_maps.append({
            "xT": np.ascontiguousarray(xpad[c * R:(c + 1) * R].T).astype(np.float16),
            "idx": idx_d[c], "slotp": slotp_d[c], "invdm": invd_d[c],
            "W0": np.ascontiguousarray(W0.astype(np.float16)),
            "W1": np.ascontiguousarray(W1.astype(np.float16)),
            "W2p": W2p,
            "b0": np.asarray(b0, dtype=np.float32).reshape(F, 1),
            "b1": np.asarray(b1, dtype=np.float32).reshape(F, 1),
            "b2p": b2p,
            "iota16": iota16,
        })
    sched = (tuple(npairs.tolist()), tuple(pair_tile.tolist()),
             tuple(pair_base.tolist()), T, NP)
    return in_maps, sched


def _build(sched):
    npairs, pair_tile, pair_base, T, NP = sched

    nc = bacc.Bacc("TRN2", num_devices=NCORES, dynamic_dma_scratch_size=65536)
    dt = mybir.dt.float32
    f16 = mybir.dt.float16

    xT_in = nc.dram_tensor("xT", [IN_FEATS, R], f16, kind="ExternalInput")
    idx_in = nc.dram_tensor("idx", [128, T * 8], mybir.dt.int16, kind="ExternalInput")
    slot_in = nc.dram_tensor("slotp", [128, NP], dt, kind="ExternalInput")
    invd_in = nc.dram_tensor("invdm", [128, T], dt, kind="ExternalInput")
    W0_in = nc.dram_tensor("W0", [IN_FEATS, F], f16, kind="ExternalInput")
    W1_in = nc.dram_tensor("W1", [F, F], f16, kind="ExternalInput")
    W2_in = nc.dram_tensor("W2p", [F, F], f16, kind="ExternalInput")
    b0_in = nc.dram_tensor("b0", [F, 1], dt, kind="ExternalInput")
    b1_in = nc.dram_tensor("b1", [F, 1], dt, kind="ExternalInput")
    b2_in = nc.dram_tensor("b2p", [F, 1], dt, kind="ExternalInput")
    iota_in = nc.dram_tensor("iota16", [128, 128], f16, kind="ExternalInput")
    out = nc.dram_tensor("out", [BPC, N_CLASSES, 128], dt, kind="ExternalOutput")

    NSEG = len(JS) - 1

    with tile.TileContext(nc) as tc:
        with tc.tile_pool(name="const", bufs=1) as cp, \
             tc.tile_pool(name="dram", bufs=1, space="DRAM") as dram, \
             tc.tile_pool(name="msg", bufs=3) as mp, \
             tc.tile_pool(name="msg16", bufs=3) as m16p, \
             tc.tile_pool(name="stl", bufs=10) as sp, \
             tc.tile_pool(name="xbp", bufs=2) as xbp, \
             tc.tile_pool(name="po", bufs=3) as pop, \
             tc.tile_pool(name="ep", bufs=2) as epp, \
             tc.tile_pool(name="agg", bufs=2, space="PSUM") as pp, \
             tc.tile_pool(name="eps", bufs=2, space="PSUM") as pp2:

            W0_t = cp.tile([IN_FEATS, F], f16)
            nc.sync.dma_start(W0_t[:], W0_in[:])
            W1_t = cp.tile([F, F], f16)
            nc.sync.dma_start(W1_t[:], W1_in[:])
            W2_t = cp.tile([F, F], f16)
            nc.sync.dma_start(W2_t[:], W2_in[:])
            b0_t = cp.tile([F, 1], dt)
            nc.sync.dma_start(b0_t[:], b0_in[:])
            b1_t = cp.tile([F, 1], dt)
            nc.sync.dma_start(b1_t[:], b1_in[:])
            b2_t = cp.tile([F, 1], dt)
            nc.sync.dma_start(b2_t[:], b2_in[:])
            iota_t = cp.tile([128, 128], f16)
            nc.sync.dma_start(iota_t[:], iota_in[:])
            idx_t = cp.tile([128, T * 8], mybir.dt.int16)
            nc.sync.dma_start(idx_t[:], idx_in[:])
            slot_t = cp.tile([128, NP], dt)
            nc.sync.dma_start(slot_t[:], slot_in[:])
            invd_t = cp.tile([128, T], dt)
            nc.sync.dma_start(invd_t[:], invd_in[:])

            tbl = [dram.tile([R, F], dt, tag=f"t{l}", name=f"t{l}") for l in range(3)]
            parts, rss = [], []
            for l in range(3):
                w = F if l < 2 else N_CLASSES
                parts.append([dram.tile([NCORES, JS[s + 1] - JS[s], w, 128], f16,
                                        tag=f"p{l}s{s}", name=f"p{l}s{s}")
                              for s in range(NSEG)])
                rss.append([dram.tile([JS[s + 1] - JS[s], w, 128], f16,
                                      tag=f"rs{l}s{s}", name=f"rs{l}s{s}")
                            for s in range(NSEG)])

            # ---- initial projection: t0 = X_c @ W0, batched 8 blocks ----
            for k0 in range(0, BPC, 8):
                g = min(8, BPC - k0)
                xb = xbp.tile([IN_FEATS, 8, 128], f16, tag="xb")
                nc.sync.dma_start(xb[:, 0:g, :].rearrange("f g s -> f (g s)"),
                                  xT_in[:, k0 * 128:(k0 + g) * 128])
                pj = pp2.tile([128, 8, F], dt, tag="pj8")
                for i in range(g):
                    nc.tensor.matmul(pj[:, i, :], xb[:, i, :], W0_t[:],
                                     start=True, stop=True)
                ys = epp.tile([128, 8, F], dt, tag="ys8")
                nc.scalar.activation(ys[:, 0:g, :], pj[:, 0:g, :],
                                     mybir.ActivationFunctionType.Identity)
                nc.sync.dma_start(
                    tbl[0][k0 * 128:(k0 + g) * 128, :].rearrange(
                        "(g p) f -> p g f", g=g), ys[:, 0:g, :])

            # ---- layers ----
            def epilogue(l, seg):
                jlo, jhi = JS[seg], JS[seg + 1]
                rs = rss[l][seg]
                if l < 2:
                    bias = b0_t if l == 0 else b1_t
                    Wn = W1_t if l == 0 else W2_t
                    for k0 in range(jlo, jhi, 8):
                        g = min(8, jhi - k0)
                        a16 = epp.tile([F, 8, 128], f16, tag="a16")
                        nc.sync.dma_start(
                            a16[:, 0:g, :],
                            rs[k0 - jlo:k0 - jlo + g].rearrange("g f s -> f g s"))
                        hT = epp.tile([F, 8, 128], f16, tag="hT")
                        nc.scalar.activation(hT[:, 0:g, :], a16[:, 0:g, :],
                                             mybir.ActivationFunctionType.Relu,
                                             bias=bias[:, 0:1])
                        y8 = pp2.tile([128, 8, F], dt, tag="pj8")
                        for i in range(g):
                            nc.tensor.matmul(y8[:, i, :], hT[:, i, :], Wn[:],
                                             start=True, stop=True)
                        ys = epp.tile([128, 8, F], dt, tag="ys8")
                        nc.scalar.activation(ys[:, 0:g, :], y8[:, 0:g, :],
                                             mybir.ActivationFunctionType.Identity)
                        nc.sync.dma_start(
                            tbl[l + 1][k0 * 128:(k0 + g) * 128, :].rearrange(
                                "(g p) f -> p g f", g=g), ys[:, 0:g, :])
                else:
                    for k0 in range(jlo, jhi, 8):
                        g = min(8, jhi - k0)
                        o16 = epp.tile([N_CLASSES, 8, 128], f16, tag="o16")
                        nc.sync.dma_start(
                            o16[:, 0:g, :],
                            rs[k0 - jlo:k0 - jlo + g].rearrange("g f s -> f g s"))
                        of = epp.tile([N_CLASSES, 8, 128], dt, tag="of")
                        nc.scalar.activation(of[:, 0:g, :], o16[:, 0:g, :],
                                             mybir.ActivationFunctionType.Identity)
                        nc.sync.dma_start(
                            out[k0:k0 + g].rearrange("g f s -> f g s"),
                            of[:, 0:g, :])

            for l in range(3):
                table = tbl[l]
                ph_f = F if l < 2 else N_CLASSES
                msgs = []

                def emit_chunk():
                    ch = len(msgs)
                    nt = min(CH, T - ch * CH)
                    msg = mp.tile([128, CH, F], dt, tag="msg")
                    nc.gpsimd.dma_gather(
                        msg[:, 0:nt, :], table[:],
                        idx_t[:, ch * CH * 8: ch * CH * 8 + nt * 8],
                        num_idxs=nt * 128, num_idxs_reg=nt * 128,
                        elem_size=F, single_packet=False)
                    msg16 = m16p.tile([128, CH, F], f16, tag="msg16")
                    nc.scalar.activation(msg16[:, 0:nt, :], msg[:, 0:nt, :],
                                         mybir.ActivationFunctionType.Identity)
                    msgs.append(msg16)

                pend = []
                for j in range(BPC):
                    for s in range(NSEG):
                        if j == JS[s + 1]:      # segment s complete -> RS
                            nc.gpsimd.collective_compute(
                                "ReduceScatter", mybir.AluOpType.add,
                                replica_groups=[list(range(NCORES))],
                                ins=[parts[l][s][:]], outs=[rss[l][s][:]])
                            pend.append(s)
                        if j == JS[s + 1] + 11 and pend:
                            epilogue(l, pend.pop(0))
                    seg = next(s for s in range(NSEG) if JS[s] <= j < JS[s + 1])
                    agg = pp.tile([F, 8, 128], dt, tag="agg")
                    for c in range(NCORES):
                        b = BPC * c + j
                        np_b = int(npairs[b])
                        for i in range(np_b):
                            p = int(pair_base[b]) + i
                            t = int(pair_tile[p])
                            while t >= len(msgs) * CH:
                                emit_chunk()
                            S = sp.tile([128, 128], f16, tag="S")
                            nc.vector.tensor_scalar(
                                S[:], iota_t[:], slot_t[:, p:p + 1],
                                invd_t[:, t:t + 1],
                                mybir.AluOpType.is_equal, mybir.AluOpType.mult)
                            nc.tensor.matmul(
                                agg[:, c, :], msgs[t // CH][:, t % CH, :], S[:],
                                start=(i == 0), stop=(i == np_b - 1))
                    po = pop.tile([ph_f, NCORES, 128], f16, tag="po")
                    if l < 2:
                        nc.scalar.activation(po[:], agg[:],
                                             mybir.ActivationFunctionType.Identity)
                    else:
                        nc.scalar.activation(po[:], agg[0:N_CLASSES, :, :],
                                             mybir.ActivationFunctionType.Identity,
                                             bias=b2_t[0:N_CLASSES, 0:1])
                    nc.sync.dma_start(
                        parts[l][seg][:, j - JS[seg]].rearrange("g f s -> f g s"),
                        po[:])

                # final segment RS + remaining epilogues
                nc.gpsimd.collective_compute(
                    "ReduceScatter", mybir.AluOpType.add,
                    replica_groups=[list(range(NCORES))],
                    ins=[parts[l][NSEG - 1][:]], outs=[rss[l][NSEG - 1][:]])
                for s in pend:
                    epilogue(l, s)
                epilogue(l, NSEG - 1)

    nc.compile()
    return nc


def kernel(features, src, dst, W0, b0, W1, b1, W2, b2):
    features = np.asarray(features, dtype=np.float32)
    src = np.asarray(src).astype(np.int64)
    dst = np.asarray(dst).astype(np.int64)
    in_maps, sched = _prep(features, src, dst,
                           np.asarray(W0), np.asarray(b0), np.asarray(W1),
                           np.asarray(b1), np.asarray(W2), np.asarray(b2))
    if _cache.get("key") != sched:
        _cache["nc"] = _build(sched)
        _cache["key"] = sched
    nc = _cache["nc"]
    res = run_bass_kernel_spmd(nc, in_maps, core_ids=list(range(NCORES)))
    shards = []
    for c in range(NCORES):
        o = res.results[c]["out"]                      # [BPC, 40, 128]
        shards.append(o.transpose(0, 2, 1).reshape(R, N_CLASSES))
    full = np.concatenate(shards, axis=0)
    return np.ascontiguousarray(full[:N_NODES])


# revision 12
# speedup vs baseline: 5.8133x; 1.0211x over previous
"""GCN (3-layer GraphConv, norm='right') Trainium2 Bass kernel — 8-core SPMD.

Sharding: nodes are split into 8 contiguous shards of R=6272 rows (padded to
50176 = 392 blocks of 128). Core c owns rows [cR,(c+1)R): it holds that shard
of the (projected) feature table and processes every edge whose SRC lies in
its shard, so all gathers are local. Each core accumulates partial aggregates
for ALL dst blocks (one-hot S-matmuls into PSUM, inv_deg folded into S, done
transposed: stationary=msg, moving=S, so partials are feature-major and no
PE transposes are ever needed); ReduceScatter(add) over the 8 cores then
hands core c the complete aggregate for its own 49 dst blocks. The epilogue
applies bias+ReLU and the next layer's projection, writing the next local
table. Layer-3's table is pre-projected by W2 so its aggregation directly
yields logits; b2/8 is added on every core so the RS sum restores b2.

Edge layout (straddle): blocks are processed j-major (round j = blocks 49c+j
for all c) and each block's edge region is sized max_c cnt[c,b] — regions are
packed contiguously WITHOUT per-block 128-alignment, so a 128-edge gather
tile may straddle adjacent block regions; each (tile, block-overlap) gets its
own one-hot S matmul (pair). This keeps gather padding at ~9% instead of
~50%. Gathered rows cost one DMA descriptor each (~1.42ns effective), making
rows-gathered the dominant term; everything else (S-builds on DVE, fp16
matmuls on PE, copies on ACT, partial writes) overlaps under it.

The per-layer partial table is split into three j-ranges so the first two
ReduceScatters (and their epilogues) overlap the remaining aggregation; only
the last small RS sits on the layer tail. All small DMAs are batched 8
blocks at a time (HWDGE fixed cost ~625ns each); PSUM holds 8 aggregation
targets per 2-bank tile so one ACT copy moves a whole round. Region sizes
are equalized across cores (max over cores) so all 8 cores run the identical
program (SPMD); padding edges carry slot=999 and contribute exactly zero.
"""
import numpy as np

import concourse.bass as bass
import concourse.tile as tile
from concourse import bacc, mybir
from concourse.bass_utils import run_bass_kernel_spmd

N_NODES = 50000
N_EDGES = 800000
IN_FEATS, F, N_CLASSES = 128, 64, 40
NCORES = 8
NBLK = 392                      # dst blocks of 128 rows
NROWS = NBLK * 128              # 50176
BPC = NBLK // NCORES            # 49 blocks per core
R = BPC * 128                   # 6272 rows per core
CH = 48                         # gather chunk size in tiles (128 idxs each)
JS = (0, 22, 38, 46, BPC)       # ReduceScatter split points (rounds)

_cache = {}


def _prep(features, src, dst, W0, b0, W1, b1, W2, b2):
    deg = np.bincount(dst, minlength=N_NODES).astype(np.float32)
    invd = (1.0 / np.maximum(deg, 1.0)).astype(np.float32)

    core = src // R
    blk = dst // 128
    cnt = np.zeros((NCORES, NBLK), dtype=np.int64)
    np.add.at(cnt, (core, blk), 1)
    nb = np.maximum(cnt.max(axis=0), 1)              # region size per block

    # processing order: j-major
    order = np.array([BPC * c + j for j in range(BPC) for c in range(NCORES)])
    estart = np.zeros(NBLK, dtype=np.int64)
    estart[order] = np.concatenate([[0], np.cumsum(nb[order])[:-1]])
    E_tot = int(nb.sum())
    T = (E_tot + 127) // 128

    t_lo = estart // 128
    t_hi = (estart + nb - 1) // 128
    npairs = (t_hi - t_lo + 1).astype(np.int64)
    pair_base = np.zeros(NBLK, dtype=np.int64)
    pair_base[order] = np.concatenate([[0], np.cumsum(npairs[order])[:-1]])
    NP = int(npairs.sum())
    pair_tile = np.zeros(NP, dtype=np.int64)
    for b in range(NBLK):
        pair_tile[pair_base[b]:pair_base[b] + npairs[b]] = np.arange(
            t_lo[b], t_hi[b] + 1)

    idx_d, slotp_d, invd_d = {}, {}, {}
    for c in range(NCORES):
        m = core == c
        s_c = src[m] - c * R
        d_c = dst[m]
        b_c = blk[m]
        o = np.argsort(b_c, kind="stable")
        s_c, d_c, b_c = s_c[o], d_c[o], b_c[o]
        cc = cnt[c]
        starts = np.concatenate([[0], np.cumsum(cc)[:-1]])
        rank = np.arange(len(s_c)) - np.repeat(starts, cc)
        pos = estart[b_c] + rank
        tile_e = pos // 128
        lane = pos % 128

        idx_pad = np.zeros(T * 128, dtype=np.int16)
        invd_pad = np.zeros(T * 128, dtype=np.float32)
        idx_pad[pos] = s_c.astype(np.int16)
        invd_pad[pos] = invd[d_c]
        slotp = np.full((128, NP), 999.0, dtype=np.float32)
        slotp[lane, pair_base[b_c] + tile_e - t_lo[b_c]] = (
            d_c % 128).astype(np.float32)

        idx_d[c] = np.tile(idx_pad.reshape(-1, 16).T, (8, 1)).copy()  # [128,T*8]
        invd_d[c] = np.ascontiguousarray(invd_pad.reshape(T, 128).T)  # [128,T]
        slotp_d[c] = slotp

    W2p = np.zeros((F, F), dtype=np.float16)
    W2p[:, :N_CLASSES] = W2[:, :N_CLASSES].astype(np.float16)
    b2p = np.zeros((F, 1), dtype=np.float32)
    b2p[:min(len(b2), F), 0] = np.asarray(b2).reshape(-1)[:F] / NCORES

    xpad = np.zeros((NROWS, IN_FEATS), dtype=np.float32)
    xpad[:N_NODES] = features
    iota16 = np.tile(np.arange(128, dtype=np.float16), (128, 1))

    in_maps = []
    for c in range(NCORES):
        in_maps.append({
            "xT": np.ascontiguousarray(xpad[c * R:(c + 1) * R].T).astype(np.float16),
            "idx": idx_d[c], "slotp": slotp_d[c], "invdm": invd_d[c],
            "W0": np.ascontiguousarray(W0.astype(np.float16)),
            "W1": np.ascontiguousarray(W1.astype(np.float16)),
            "W2p": W2p,
            "b0": np.asarray(b0, dtype=np.float32).reshape(F, 1),
            "b1": np.asarray(b1, dtype=np.float32).reshape(F, 1),
            "b2p": b2p,
            "iota16": iota16,
        })
    sched = (tuple(npairs.tolist()), tuple(pair_tile.tolist()),
             tuple(pair_base.tolist()), T, NP)
    return in_maps, sched


def _build(sched):
    npairs, pair_tile, pair_base, T, NP = sched

    nc = bacc.Bacc("TRN2", num_devices=NCORES, dynamic_dma_scratch_size=65536)
    dt = mybir.dt.float32
    f16 = mybir.dt.float16

    xT_in = nc.dram_tensor("xT", [IN_FEATS, R], f16, kind="ExternalInput")
    idx_in = nc.dram_tensor("idx", [128, T * 8], mybir.dt.int16, kind="ExternalInput")
    slot_in = nc.dram_tensor("slotp", [128, NP], dt, kind="ExternalInput")
    invd_in = nc.dram_tensor("invdm", [128, T], dt, kind="ExternalInput")
    W0_in = nc.dram_tensor("W0", [IN_FEATS, F], f16, kind="ExternalInput")
    W1_in = nc.dram_tensor("W1", [F, F], f16, kind="ExternalInput")
    W2_in = nc.dram_tensor("W2p", [F, F], f16, kind="ExternalInput")
    b0_in = nc.dram_tensor("b0", [F, 1], dt, kind="ExternalInput")
    b1_in = nc.dram_tensor("b1", [F, 1], dt, kind="ExternalInput")
    b2_in = nc.dram_tensor("b2p", [F, 1], dt, kind="ExternalInput")
    iota_in = nc.dram_tensor("iota16", [128, 128], f16, kind="ExternalInput")
    out = nc.dram_tensor("out", [BPC, N_CLASSES, 128], dt, kind="ExternalOutput")

    NSEG = len(JS) - 1

    with tile.TileContext(nc) as tc:
        with tc.tile_pool(name="const", bufs=1) as cp, \
             tc.tile_pool(name="dram", bufs=1, space="DRAM") as dram, \
             tc.tile_pool(name="msg", bufs=3) as mp, \
             tc.tile_pool(name="msg16", bufs=3) as m16p, \
             tc.tile_pool(name="stl", bufs=10) as sp, \
             tc.tile_pool(name="xbp", bufs=2) as xbp, \
             tc.tile_pool(name="po", bufs=3) as pop, \
             tc.tile_pool(name="ep", bufs=2) as epp, \
             tc.tile_pool(name="agg", bufs=2, space="PSUM") as pp, \
             tc.tile_pool(name="eps", bufs=2, space="PSUM") as pp2:

            W0_t = cp.tile([IN_FEATS, F], f16)
            nc.sync.dma_start(W0_t[:], W0_in[:])
            W1_t = cp.tile([F, F], f16)
            nc.sync.dma_start(W1_t[:], W1_in[:])
            W2_t = cp.tile([F, F], f16)
            nc.sync.dma_start(W2_t[:], W2_in[:])
            b0_t = cp.tile([F, 1], dt)
            nc.sync.dma_start(b0_t[:], b0_in[:])
            b1_t = cp.tile([F, 1], dt)
            nc.sync.dma_start(b1_t[:], b1_in[:])
            b2_t = cp.tile([F, 1], dt)
            nc.sync.dma_start(b2_t[:], b2_in[:])
            iota_t = cp.tile([128, 128], f16)
            nc.sync.dma_start(iota_t[:], iota_in[:])
            idx_t = cp.tile([128, T * 8], mybir.dt.int16)
            nc.sync.dma_start(idx_t[:], idx_in[:])
            slot_t = cp.tile([128, NP], dt)
            nc.sync.dma_start(slot_t[:], slot_in[:])
            invd_t = cp.tile([128, T], dt)
            nc.sync.dma_start(invd_t[:], invd_in[:])

            # partial tables are pair-packed [c, j//2, w, 2, 128] so DMA
            # descriptors are 512B (256B descriptors pay a 2x latency penalty)
            tbl = [dram.tile([R, F], dt, tag=f"t{l}", name=f"t{l}") for l in range(3)]
            parts, rss = [], []
            for l in range(3):
                w = F if l < 2 else N_CLASSES
                parts.append([dram.tile(
                    [NCORES, (JS[s + 1] - JS[s] + 1) // 2, w, 2, 128], f16,
                    tag=f"p{l}s{s}", name=f"p{l}s{s}") for s in range(NSEG)])
                rss.append([dram.tile(
                    [(JS[s + 1] - JS[s] + 1) // 2, w, 2, 128], f16,
                    tag=f"rs{l}s{s}", name=f"rs{l}s{s}") for s in range(NSEG)])

            # ---- initial projection: t0 = X_c @ W0, batched 8 blocks ----
            for k0 in range(0, BPC, 8):
                g = min(8, BPC - k0)
                xb = xbp.tile([IN_FEATS, 8, 128], f16, tag="xb")
                nc.sync.dma_start(xb[:, 0:g, :].rearrange("f g s -> f (g s)"),
                                  xT_in[:, k0 * 128:(k0 + g) * 128])
                pj = pp2.tile([128, 8, F], dt, tag="pj8")
                for i in range(g):
                    nc.tensor.matmul(pj[:, i, :], xb[:, i, :], W0_t[:],
                                     start=True, stop=True)
                ys = epp.tile([128, 8, F], dt, tag="ys8")
                nc.scalar.activation(ys[:, 0:g, :], pj[:, 0:g, :],
                                     mybir.ActivationFunctionType.Identity)
                nc.sync.dma_start(
                    tbl[0][k0 * 128:(k0 + g) * 128, :].rearrange(
                        "(g p) f -> p g f", g=g), ys[:, 0:g, :])

            # ---- layers ----
            def epilogue(l, seg):
                jlo, jhi = JS[seg], JS[seg + 1]
                rs = rss[l][seg]
                if l < 2:
                    bias = b0_t if l == 0 else b1_t
                    Wn = W1_t if l == 0 else W2_t
                    for k0 in range(jlo, jhi, 8):
                        g = min(8, jhi - k0)
                        npr = (g + 1) // 2
                        p0 = (k0 - jlo) // 2
                        a16 = epp.tile([F, 8, 128], f16, tag="a16")
                        nc.sync.dma_start(
                            a16[:, 0:2 * npr, :].rearrange("f (p j) s -> f p j s",
                                                           j=2),
                            rs[p0:p0 + npr].rearrange("p f j s -> f p j s"))
                        hT = epp.tile([F, 8, 128], f16, tag="hT")
                        nc.scalar.activation(hT[:, 0:g, :], a16[:, 0:g, :],
                                             mybir.ActivationFunctionType.Relu,
                                             bias=bias[:, 0:1])
                        y8 = pp2.tile([128, 8, F], dt, tag="pj8")
                        for i in range(g):
                            nc.tensor.matmul(y8[:, i, :], hT[:, i, :], Wn[:],
                                             start=True, stop=True)
                        ys = epp.tile([128, 8, F], dt, tag="ys8")
                        nc.scalar.activation(ys[:, 0:g, :], y8[:, 0:g, :],
                                             mybir.ActivationFunctionType.Identity)
                        nc.sync.dma_start(
                            tbl[l + 1][k0 * 128:(k0 + g) * 128, :].rearrange(
                                "(g p) f -> p g f", g=g), ys[:, 0:g, :])
                else:
                    for k0 in range(jlo, jhi, 8):
                        g = min(8, jhi - k0)
                        npr = (g + 1) // 2
                        p0 = (k0 - jlo) // 2
                        o16 = epp.tile([N_CLASSES, 8, 128], f16, tag="o16")
                        nc.sync.dma_start(
                            o16[:, 0:2 * npr, :].rearrange("f (p j) s -> f p j s",
                                                           j=2),
                            rs[p0:p0 + npr].rearrange("p f j s -> f p j s"))
                        of = epp.tile([N_CLASSES, 8, 128], dt, tag="of")
                        nc.scalar.activation(of[:, 0:g, :], o16[:, 0:g, :],
                                             mybir.ActivationFunctionType.Identity)
                        nc.sync.dma_start(
                            out[k0:k0 + g].rearrange("g f s -> f g s"),
                            of[:, 0:g, :])

            for l in range(3):
                table = tbl[l]
                ph_f = F if l < 2 else N_CLASSES
                msgs = []

                def emit_chunk():
                    ch = len(msgs)
                    nt = min(CH, T - ch * CH)
                    msg = mp.tile([128, CH, F], dt, tag="msg")
                    nc.gpsimd.dma_gather(
                        msg[:, 0:nt, :], table[:],
                        idx_t[:, ch * CH * 8: ch * CH * 8 + nt * 8],
                        num_idxs=nt * 128, num_idxs_reg=nt * 128,
                        elem_size=F, single_packet=False)
                    msg16 = m16p.tile([128, CH, F], f16, tag="msg16")
                    nc.scalar.activation(msg16[:, 0:nt, :], msg[:, 0:nt, :],
                                         mybir.ActivationFunctionType.Identity)
                    msgs.append(msg16)

                pend = []
                po2 = None
                for j in range(BPC):
                    for s in range(NSEG):
                        if j == JS[s + 1]:      # segment s complete -> RS
                            nc.gpsimd.collective_compute(
                                "ReduceScatter", mybir.AluOpType.add,
                                replica_groups=[list(range(NCORES))],
                                ins=[parts[l][s][:]], outs=[rss[l][s][:]])
                            pend.append(s)
                        if j == JS[s + 1] + 11 and pend:
                            epilogue(l, pend.pop(0))
                    seg = next(s for s in range(NSEG) if JS[s] <= j < JS[s + 1])
                    agg = pp.tile([F, 8, 128], dt, tag="agg")
                    for c in range(NCORES):
                        b = BPC * c + j
                        np_b = int(npairs[b])
                        for i in range(np_b):
                            p = int(pair_base[b]) + i
                            t = int(pair_tile[p])
                            while t >= len(msgs) * CH:
                                emit_chunk()
                            S = sp.tile([128, 128], f16, tag="S")
                            nc.vector.tensor_scalar(
                                S[:], iota_t[:], slot_t[:, p:p + 1],
                                invd_t[:, t:t + 1],
                                mybir.AluOpType.is_equal, mybir.AluOpType.mult)
                            nc.tensor.matmul(
                                agg[:, c, :], msgs[t // CH][:, t % CH, :], S[:],
                                start=(i == 0), stop=(i == np_b - 1))
                    q = (j - JS[seg]) % 2
                    if q == 0:
                        po2 = pop.tile([ph_f, NCORES, 2, 128], f16, tag="po")
                    if l < 2:
                        nc.scalar.activation(po2[:, :, q, :], agg[:],
                                             mybir.ActivationFunctionType.Identity)
                    else:
                        nc.scalar.activation(po2[:, :, q, :], agg[0:N_CLASSES, :, :],
                                             mybir.ActivationFunctionType.Identity,
                                             bias=b2_t[0:N_CLASSES, 0:1])
                    if q == 1 or j == JS[seg + 1] - 1:
                        jp = (j - JS[seg]) // 2
                        nc.sync.dma_start(
                            parts[l][seg][:, jp, :, 0:q + 1, :].rearrange(
                                "g f j s -> f g j s"),
                            po2[:, :, 0:q + 1, :])

                # final segment RS + remaining epilogues
                nc.gpsimd.collective_compute(
                    "ReduceScatter", mybir.AluOpType.add,
                    replica_groups=[list(range(NCORES))],
                    ins=[parts[l][NSEG - 1][:]], outs=[rss[l][NSEG - 1][:]])
                for s in pend:
                    epilogue(l, s)
                epilogue(l, NSEG - 1)

    nc.compile()
    return nc


def kernel(features, src, dst, W0, b0, W1, b1, W2, b2):
    features = np.asarray(features, dtype=np.float32)
    src = np.asarray(src).astype(np.int64)
    dst = np.asarray(dst).astype(np.int64)
    in_maps, sched = _prep(features, src, dst,
                           np.asarray(W0), np.asarray(b0), np.asarray(W1),
                           np.asarray(b1), np.asarray(W2), np.asarray(b2))
    if _cache.get("key") != sched:
        _cache["nc"] = _build(sched)
        _cache["key"] = sched
    nc = _cache["nc"]
    res = run_bass_kernel_spmd(nc, in_maps, core_ids=list(range(NCORES)))
    shards = []
    for c in range(NCORES):
        o = res.results[c]["out"]                      # [BPC, 40, 128]
        shards.append(o.transpose(0, 2, 1).reshape(R, N_CLASSES))
    full = np.concatenate(shards, axis=0)
    return np.ascontiguousarray(full[:N_NODES])
